# revision 1
# baseline (speedup 1.0000x reference)
# Trainium2 Bass kernel for nn_Decoder (LSTM decoder w/ Luong attention + vocab projection)
#
# Sharding: 8 cores = 2 batch-groups x 4 vocab-slices.
#   Each core runs the full recurrence for its 32-batch group (replicated across
#   the 4 vocab-slice cores of that group) and computes logits for its 8000-wide
#   vocab slice, streamed out to HBM during the recurrence.
#
# Numerics: all recurrence-loop matmuls are exact fp32 (4-way PE column-group
#   packing keeps them fast at M=32); scores use fp32r on explicitly-rounded
#   twins; context/fc use fp32r (one-shot error only).
# Device activations use a single ACT table set (tanh/exp):
#   sigmoid(x) = (1 + tanh(x/2)) / 2 ; state kept doubled: ct = 2c, ht = 2h;
#   all 0.5 compensations folded host-side into weights (see _prep).
# Layouts:
#   cell state lives u-split: partition (32q + b) holds batch b, u-quarter q.
#   h/attn are re-transposed to [64, (q, b)] "T-form" and consumed as K=64
#   matmul chunks (PE transpose outputs must start at PSUM partition 0).
# Attention: scoresT[(b,s), b] via keysT (K=64 chunks) x rounded-h, exp +
#   constant block-diag mask -> bd; context + softmax denominator in one
#   accumulated matmul against mem_pad augmented with a ones column.
import numpy as np
from contextlib import ExitStack

import concourse.bass as bass
import concourse.bacc as bacc
import concourse.mybir as mybir
import concourse.tile as tile
from concourse import bass_utils

B, T, S, E, U, V = 64, 30, 31, 128, 256, 32000
NB, NV = 2, 4                  # batch groups x vocab slices = 8 cores
BL, VL = B // NB, V // NV      # 32 local batch, 8000 local vocab
SP = 32                        # padded source length
NCH = (BL * SP) // 128         # 8 block-diag / (b,s) chunks
UC = U + 4                     # context matmul width (col U = softmax denom)
FCB = 500                      # fc psum bank width (<=512)
NFCB = VL // FCB               # 16
MROWS = T * BL                 # 960 fc rows, (t,b) order
F32 = mybir.dt.float32
AF = mybir.ActivationFunctionType
ALU = mybir.AluOpType
AX = mybir.AxisListType


def _r(ap):
    """fp32r view (PE rounds operands to ~12-bit mantissa, full rate N>=256)."""
    return ap.bitcast(mybir.dt.float32r)


def _ap(t, offset, dims):
    """Custom access pattern on a tile (flat element space)."""
    return bass.AP(t.tensor, t.offset + offset, dims)


def build_program(has_blstm: bool):
    nc = bacc.Bacc("TRN2", target_bir_lowering=False, debug=False,
                   enable_asserts=False, num_devices=NB * NV)
    f = F32
    mmr = lambda out, lhsT, rhs, **kw: nc.tensor.matmul(out, _r(lhsT), _r(rhs), **kw)
    mmf = lambda out, lhsT, rhs, **kw: nc.tensor.matmul(out, lhsT, rhs, **kw)
    # ---- per-core external I/O ----
    xT = nc.dram_tensor("xT", [E, T * BL], f, kind="ExternalInput").ap()
    wbigx = nc.dram_tensor("wbigx", [128, 4 * U], f, kind="ExternalInput").ap()
    wbigah = nc.dram_tensor("wbigah", [8, 64, 4 * U], f, kind="ExternalInput").ap()
    wattnh = nc.dram_tensor("wattnh", [4, 64, U], f, kind="ExternalInput").ap()
    wattnc = nc.dram_tensor("wattnc", [2, 128, U], f, kind="ExternalInput").ap()
    wfc = nc.dram_tensor("wfc", [2, 128, VL], f, kind="ExternalInput").ap()
    wmem = nc.dram_tensor("wmem", [2, 128, U], f, kind="ExternalInput").ap()
    memp = nc.dram_tensor("memp", [128, NCH * UC], f, kind="ExternalInput").ap()
    mempT = nc.dram_tensor("mempT", [128, 2 * BL * SP], f, kind="ExternalInput").ap()
    mask = nc.dram_tensor("mask", [128, NCH * BL], f, kind="ExternalInput").ap()
    h0T = nc.dram_tensor("h0T", [64, 2 * 128], f, kind="ExternalInput").ap()
    c0 = nc.dram_tensor("c0", [128, 64], f, kind="ExternalInput").ap()
    eye = nc.dram_tensor("eye", [128, 128], f, kind="ExternalInput").ap()
    perm = nc.dram_tensor("perm", [64, 2 * 128], f, kind="ExternalInput").ap()
    if has_blstm:
        blstm = nc.dram_tensor("blstm", [1, 4 * U], f, kind="ExternalInput").ap()
        onesd = nc.dram_tensor("onesd", [1, BL], f, kind="ExternalInput").ap()
    out = nc.dram_tensor("out", [MROWS, VL], f, kind="ExternalOutput").ap()

    with tile.TileContext(nc) as tc, ExitStack() as ctx:
        const = ctx.enter_context(tc.tile_pool(name="const", bufs=1))
        work = ctx.enter_context(tc.tile_pool(name="work", bufs=2))
        ps_big = ctx.enter_context(tc.tile_pool(name="ps_big", bufs=1, space="PSUM"))
        ps_sm = ctx.enter_context(tc.tile_pool(name="ps_sm", bufs=2, space="PSUM"))
        ps_tp = ctx.enter_context(tc.tile_pool(name="ps_tp", bufs=2, space="PSUM"))

        # ---- load constants into SBUF ----
        xTs = const.tile([E, T * BL], f)
        nc.sync.dma_start(xTs[:], xT[:])
        wbigXS = const.tile([128, 4 * U], f)
        nc.sync.dma_start(wbigXS[:], wbigx[:])
        wbigAHS = const.tile([64, 8, 4 * U], f)
        nc.sync.dma_start(wbigAHS[:], wbigah.transpose([1, 0, 2]))
        wattnHS = const.tile([64, 4, U], f)
        nc.sync.dma_start(wattnHS[:], wattnh.transpose([1, 0, 2]))
        wattnCS = const.tile([128, 2, U], f)
        nc.sync.dma_start(wattnCS[:], wattnc.transpose([1, 0, 2]))
        wfcS = const.tile([128, 2, VL], f)
        nc.sync.dma_start(_r(wfcS[:]), _r(wfc.transpose([1, 0, 2])))
        wmemS = const.tile([128, 2, U], f)
        nc.sync.dma_start(wmemS[:], wmem.transpose([1, 0, 2]))
        eyeS = const.tile([128, 128], f)
        nc.sync.dma_start(eyeS[:], eye[:])
        permS = const.tile([64, 2, 128], f)
        nc.sync.dma_start(_r(permS[:]), _r(perm.rearrange("p (k n) -> p k n", k=2)))
        mempS = const.tile([128, NCH, UC], f)
        nc.sync.dma_start(_r(mempS[:]), _r(memp.rearrange("p (c u) -> p c u", c=NCH)))
        mempTS = const.tile([128, 2, BL * SP], f)
        nc.sync.dma_start(mempTS[:], mempT.rearrange("p (k n) -> p k n", k=2))
        maskS = const.tile([128, NCH, BL], f)
        nc.sync.dma_start(maskS[:], mask.rearrange("p (c b) -> p c b", c=NCH))
        if has_blstm:
            blstmS = const.tile([1, 4 * U], f)
            nc.sync.dma_start(blstmS[:], blstm[:])
            onesS = const.tile([1, BL], f)
            nc.sync.dma_start(onesS[:], onesd[:])

        # keysT64 [64, 4, BL*SP]: rows u-quarters; exact fp32 build, rounded store
        keysT = const.tile([64, 4, BL * SP], f)
        for m in range(4):
            for b_ in range(2):
                kp = ps_big.tile([64, 512], f, tag="big")
                for k in range(2):
                    mmf(kp[:], wmemS[:, k, 64 * m:64 * (m + 1)],
                        mempTS[:, k, 512 * b_:512 * (b_ + 1)],
                        start=(k == 0), stop=(k == 1))
                nc.vector.tensor_copy(_r(keysT[:, m, 512 * b_:512 * (b_ + 1)]), kp[:])

        # ---- persistent state ----
        # STATE [128=(q,b), 384]: gates f,i,o,g (64 each) | ct | tc
        STATE = const.tile([128, 6 * 64], f)
        nc.sync.dma_start(STATE[:, 4 * 64:5 * 64], c0[:])
        Ts = const.tile([64, 128], f)        # h T-form [uq, (q, b)], exact
        nc.sync.dma_start(Ts[:], h0T[:, 0:128])
        Tsr = const.tile([64, 128], f)       # rounded twin for scores
        nc.sync.dma_start(_r(Tsr[:]), _r(h0T[:, 0:128]))
        Ta = const.tile([64, 128], f)        # attn T-form, exact (t-1)
        nc.sync.dma_start(Ta[:], h0T[:, 128:256])
        Tar = const.tile([64, 128], f)       # rounded twin for fc repack
        nc.sync.dma_start(_r(Tar[:]), _r(h0T[:, 128:256]))
        attn_fcT = const.tile([128, 2, MROWS], f)  # repacked (rounded) for fc

        def z_chunks(t):
            """(lhsT, rhs-selector) pairs for the z matmul; all partition-base 0."""
            ch = [(xTs[:, BL * t:BL * (t + 1)], None)]
            for a in range(4):   # attn K=64 chunks
                ch.append((Ta[:, 32 * a:32 * (a + 1)], a))
            for a in range(4):   # h K=64 chunks
                ch.append((Ts[:, 32 * a:32 * (a + 1)], 4 + a))
            return ch

        def fc_tile(m):
            r0 = 128 * m
            nrow = min(128, MROWS - r0)
            for b_ in range(NFCB):
                fp = ps_tp.tile([128, FCB], f, tag="tp")
                for k in range(2):
                    mmr(fp[0:nrow, :], attn_fcT[:, k, r0:r0 + nrow],
                        wfcS[:, k, FCB * b_:FCB * (b_ + 1)],
                        start=(k == 0), stop=(k == 1))
                fs = work.tile([128, FCB], f, tag="fs", bufs=4)
                if b_ % 2 == 0:
                    nc.vector.tensor_copy(fs[0:nrow, :], fp[0:nrow, :])
                else:
                    nc.scalar.copy(fs[0:nrow, :], fp[0:nrow, :])
                nc.sync.dma_start(out[r0:r0 + nrow, FCB * b_:FCB * (b_ + 1)],
                                  fs[0:nrow, :])

        for t in range(T):
            # --- z: exact fp32, 4 col-groups; zP[(32j+b), 256] = u-quarter j ---
            zP = ps_big.tile([128, 256], f, tag="big")
            ch = z_chunks(t)
            nk = len(ch) + (1 if has_blstm else 0)
            for j in range(4):
                oj = zP[32 * j:32 * (j + 1), :]
                for i, (lh, c) in enumerate(ch):
                    rh = (wbigXS[:, 256 * j:256 * (j + 1)] if c is None
                          else wbigAHS[:, c, 256 * j:256 * (j + 1)])
                    mmf(oj, lh, rh,
                        start=(i == 0), stop=(i == nk - 1),
                        tile_position=(0, 32 * j))
                if has_blstm:
                    mmf(oj, onesS[:], blstmS[:, 256 * j:256 * (j + 1)],
                        start=False, stop=True, tile_position=(0, 32 * j))
            # --- gates: tanh(z') -> STATE[:, 0:256] ---
            nc.scalar.activation(STATE[:, 0:4 * 64], zP[:], AF.Tanh)
            # --- cell: u=(1+tf)*ct, v=(1+ti)*tg ; ct' = 0.5u + v ---
            UV = work.tile([128, 2 * 64], f, tag="uv")
            nc.vector.scalar_tensor_tensor(
                _ap(UV[:], 0, [[128, 128], [64, 2], [1, 64]]),
                _ap(STATE[:], 0, [[384, 128], [64, 2], [1, 64]]),
                1.0,
                _ap(STATE[:], 4 * 64, [[384, 128], [-64, 2], [1, 64]]),
                op0=ALU.add, op1=ALU.mult)
            nc.vector.scalar_tensor_tensor(
                STATE[:, 4 * 64:5 * 64], UV[:, 0:64], 0.5, UV[:, 64:128],
                op0=ALU.mult, op1=ALU.add)
            nc.scalar.activation(STATE[:, 5 * 64:6 * 64], STATE[:, 4 * 64:5 * 64],
                                 AF.Tanh, scale=0.5)
            HB = work.tile([128, 64], f, tag="hb")
            nc.vector.scalar_tensor_tensor(
                HB[:], STATE[:, 2 * 64:3 * 64], 1.0, STATE[:, 5 * 64:6 * 64],
                op0=ALU.add, op1=ALU.mult)
            # --- h T-form: one transpose [128,64] -> [64,128] ---
            tph = ps_tp.tile([128, FCB], f, tag="tp")
            nc.tensor.matmul(tph[0:64, 0:128], HB[:], eyeS[:, 0:128],
                             is_transpose=True)
            nc.vector.tensor_copy(Ts[:], tph[0:64, 0:128])
            nc.vector.tensor_copy(_r(Tsr[:]), tph[0:64, 0:128])
            # --- scoresT (fp32r on rounded twins): 8 m-tiles x 4 K-chunks ---
            PS = ps_sm.tile([128, NCH, BL], f, tag="sc")
            for m in range(NCH):
                for q in range(4):
                    mmr(PS[:, m, :], keysT[:, q, 128 * m:128 * (m + 1)],
                        Tsr[:, 32 * q:32 * (q + 1)],
                        start=(q == 0), stop=(q == 3))
            ET = work.tile([128, NCH, BL], f, tag="et")
            nc.scalar.activation(ET[:], PS[:], AF.Exp)
            BD = work.tile([128, NCH, BL], f, tag="bd")
            nc.vector.tensor_mul(_r(BD[:]), ET[:], maskS[:])
            # --- context (+denominator) fp32r ---
            CX = ps_sm.tile([BL, UC], f, tag="cx")
            for c in range(NCH):
                mmr(CX[:], BD[:, c, :], mempS[:, c, :],
                    start=(c == 0), stop=(c == NCH - 1))
            rec = work.tile([BL, 1], f, tag="rc")
            nc.vector.reciprocal(rec[:], CX[:, U:U + 1])
            CXS = work.tile([BL, U], f, tag="cxs")
            nc.scalar.activation(CXS[:], CX[:, 0:U], AF.Copy, scale=rec[:])
            # ctxT [128, 2, 32] exact (transpose out base 0 is legal)
            ctxT = work.tile([128, 2, BL], f, tag="ctxT")
            for k in range(2):
                tp = ps_tp.tile([128, FCB], f, tag="tp")
                nc.tensor.matmul(tp[:, 0:BL], CXS[:, 128 * k:128 * (k + 1)],
                                 eyeS[0:BL, 0:BL], is_transpose=True)
                nc.vector.tensor_copy(ctxT[:, k, :], tp[:, 0:BL])
            # --- attn proj: exact fp32, 4 col-groups -> attn-split [(j,b), 64] ---
            AT = ps_sm.tile([128, 64], f, tag="cx")
            pch = [(Ts[:, 32 * a:32 * (a + 1)], ("h", a)) for a in range(4)]
            pch += [(ctxT[:, k, :], ("c", k)) for k in range(2)]
            for j in range(4):
                oj = AT[32 * j:32 * (j + 1), :]
                for i, (lh, sel) in enumerate(pch):
                    rh = (wattnHS[:, sel[1], 64 * j:64 * (j + 1)] if sel[0] == "h"
                          else wattnCS[:, sel[1], 64 * j:64 * (j + 1)])
                    mmf(oj, lh, rh,
                        start=(i == 0), stop=(i == len(pch) - 1),
                        tile_position=(0, 32 * j))
            ATS = work.tile([128, 64], f, tag="ats")
            nc.scalar.copy(ATS[:], AT[:])
            # attn T-form (exact, feeds next z)
            tpa = ps_tp.tile([128, FCB], f, tag="tp")
            nc.tensor.matmul(tpa[0:64, 0:128], ATS[:], eyeS[:, 0:128],
                             is_transpose=True)
            nc.vector.tensor_copy(Ta[:], tpa[0:64, 0:128])
            nc.vector.tensor_copy(_r(Tar[:]), tpa[0:64, 0:128])
            # repack attn to true [128(u), 32] via perm matmuls (fp32r, fc-only)
            for k in range(2):
                tpf = ps_tp.tile([128, FCB], f, tag="tp")
                for jj in range(2):
                    mmr(tpf[:, 0:BL], permS[:, jj, :],
                        Tar[:, 32 * (2 * k + jj):32 * (2 * k + jj) + 32],
                        start=(jj == 0), stop=(jj == 1))
                nc.vector.tensor_copy(_r(attn_fcT[:, k, BL * t:BL * (t + 1)]),
                                      tpf[:, 0:BL])
            # --- streamed fc for completed 128-row tiles ---
            if (t + 1) * BL % 128 == 0:
                fc_tile((t + 1) * BL // 128 - 1)
        if MROWS % 128 != 0:
            fc_tile(MROWS // 128)

    nc.compile()
    return nc


def _prep(inputs):
    """Host-side prep: shard + fold scales into weights. Returns in_maps list."""
    f = np.float32
    emb = np.asarray(inputs["emb_table"], f)
    W_k = np.asarray(inputs["W_k"], f)
    W_r = np.asarray(inputs["W_r"], f)
    b_l = np.asarray(inputs["b_lstm"], f)
    W_mem = np.asarray(inputs["W_mem"], f)
    W_attn = np.asarray(inputs["W_attn"], f)
    W_fc = np.asarray(inputs["W_fc"], f)
    idx_in = np.asarray(inputs["inputs"])
    memory = np.asarray(inputs["memory"], f)
    h0 = np.asarray(inputs["sample_h"], f)
    c0_ = np.asarray(inputs["sample_c"], f)

    # column permutation: per u-quarter q: [f_q*.5 | i_q*.5 | o_q*.5 | g_q]
    # jax z-split order: i [0,U), f [U,2U), g [2U,3U), o [3U,4U)
    cols, scl = [], []
    for q in range(4):
        uq = np.arange(64 * q, 64 * (q + 1))
        cols += [U + uq, 0 + uq, 3 * U + uq, 2 * U + uq]
        scl += [np.full(64, .5, f), np.full(64, .5, f),
                np.full(64, .5, f), np.ones(64, f)]
    perm_c = np.concatenate(cols)
    colscale = np.concatenate(scl)
    W_all = np.concatenate([W_k, 0.5 * W_r], axis=0)[:, perm_c] * colscale
    wbigx = np.ascontiguousarray(W_all[0:128])
    wbigah = np.ascontiguousarray(W_all[128:640].reshape(8, 64, 4 * U))
    b_p = np.ascontiguousarray((b_l[perm_c] * colscale).reshape(1, 4 * U))
    has_blstm = bool(np.any(b_p != 0))

    wattnh = np.ascontiguousarray((0.5 * W_attn[:U]).reshape(4, 64, U))
    wattnc = np.ascontiguousarray(W_attn[U:].reshape(2, 128, U))
    wmem = np.ascontiguousarray((0.5 * W_mem).reshape(2, 128, U))
    eye = np.eye(128, dtype=f)
    permm = np.ascontiguousarray(
        np.concatenate([eye[0:64], eye[64:128]], axis=1).reshape(64, 256))

    # block-diag mask: [partition 32*l+s, chunk c, col b] = (b == 4c+l and s < S)
    maskt = np.zeros((4, SP, NCH, BL), f)
    for l in range(4):
        for c in range(NCH):
            maskt[l, :S, c, 4 * c + l] = 1.0
    maskt = maskt.reshape(128, NCH * BL)

    x_emb = emb[idx_in]                      # [B, T, E] host gather
    in_maps = []
    for g in range(NB):
        bs = slice(BL * g, BL * (g + 1))
        xTl = np.ascontiguousarray(
            x_emb[bs].transpose(2, 1, 0).reshape(E, T * BL))  # cols (t, b)
        # h T-form init [uq, (q, b)] = 2h[b, 64q+uq]; attn0 = zeros
        hts = (2 * h0[bs]).reshape(BL, 4, 64).transpose(2, 1, 0).reshape(64, 128)
        h0Td = np.ascontiguousarray(
            np.concatenate([hts, np.zeros((64, 128), f)], axis=1))
        c0d = np.ascontiguousarray(
            (2 * c0_[bs]).reshape(BL, 4, 64).transpose(1, 0, 2).reshape(128, 64))
        mloc = memory[bs]                    # [BL, S, U]
        mp = np.zeros((4, SP, NCH, UC), f)
        for c in range(NCH):
            for blo in range(4):
                mp[blo, :S, c, :U] = mloc[4 * c + blo]
                mp[blo, :S, c, U] = 1.0
        mp = mp.reshape(128, NCH * UC)
        mt = np.zeros((BL, SP, U), f)
        mt[:, :S, :] = mloc
        mt = np.ascontiguousarray(
            mt.reshape(BL * SP, U).T.reshape(2, 128, BL * SP)
            .transpose(1, 0, 2).reshape(128, 2 * BL * SP))
        for v in range(NV):
            m = {"xT": xTl, "wbigx": wbigx, "wbigah": wbigah,
                 "wattnh": wattnh, "wattnc": wattnc,
                 "wfc": np.ascontiguousarray(
                     W_fc[:, VL * v:VL * (v + 1)].reshape(2, 128, VL)),
                 "wmem": wmem, "memp": mp, "mempT": mt, "mask": maskt,
                 "h0T": h0Td, "c0": c0d, "eye": eye, "perm": permm}
            if has_blstm:
                m["blstm"] = b_p
                m["onesd"] = np.ones((1, BL), f)
            in_maps.append(m)
    return in_maps, has_blstm


_CACHE = {}


def kernel(**inputs) -> np.ndarray:
    in_maps, has_blstm = _prep(inputs)
    if has_blstm not in _CACHE:
        _CACHE[has_blstm] = build_program(has_blstm)
    nc = _CACHE[has_blstm]
    res = bass_utils.run_bass_kernel_spmd(
        nc, in_maps, core_ids=list(range(NB * NV)))
    outs = [r["out"] for r in res.results]   # each [960, 8000], rows (t, b)
    full = np.empty((B, T, V), np.float32)
    for g in range(NB):
        for v in range(NV):
            o = outs[NV * g + v].reshape(T, BL, VL)
            full[BL * g:BL * (g + 1), :, VL * v:VL * (v + 1)] = o.transpose(1, 0, 2)
    b_fc = np.asarray(inputs["b_fc"], np.float32)
    if np.any(b_fc != 0):
        full = full + b_fc
    return full

